# revision 47
# baseline (speedup 1.0000x reference)
"""Grouped-Query Attention (B=2, T=2048, C=2048, 16 Q heads / 4 KV heads,
D=128) on 8 Trainium2 NeuronCores.

Sharding: core (b, g) for b in {0,1}, g in {0..3} handles batch b and KV head
g (= query heads 4g..4g+3). Each core computes its 4 heads' attention plus the
partial output projection against its 512-row slice of Wo; the host sums the
4 partials per batch (the "all-reduce" of the o_proj, done in numpy).

v2 (this file) vs baseline:
  * all matmul operands bf16 (inputs staged bf16 on host); PSUM stays fp32.
  * DMA spread across both HWDGE queues (sync + scalar engines) interleaved
    in consumption order, consts on the gpsimd SWDGE queue; kills the 56us
    startup stall and the single-queue DMA saturation.
  * RoPE runs per-t-block inside phase 1 (overlapped with projections of the
    next block) instead of as a serial phase.
  * softmax denominators batched per t-block: dens copied to one [4,512]
    sbuf tile, ONE reciprocal per t-block (was 16 x 3.3us on DVE), rescale
    via partition-broadcast AP reads (no gpsimd partition_broadcast).
  * o_proj of block tb emitted after scores of block tb' (software pipeline)
    so the PE never waits for the softmax tail.
"""
import sys

sys.path.insert(0, "/opt/trn_rl_repo")

import numpy as np

B, T, C = 2, 2048, 2048
NUM_HEADS, NUM_KV_HEADS, HEAD_DIM = 16, 4, 128
G = NUM_HEADS // NUM_KV_HEADS  # 4 query heads per core
SCALE = float(HEAD_DIM) ** -0.5
TB = 512  # t-block (matmul moving free dim)
NTB = T // TB  # 4
ST = 128  # s-tile
NST = T // ST  # 16
NCT = C // 128  # 16 contraction tiles

SWAP_MASK = [i ^ 1 for i in range(32)]

_nc_cache: dict = {}


def _classify_mask(mask2d: np.ndarray):
    """mask2d[t, s] bool. Returns (plan, masks128, masksw).

    plan[tb] is a tuple of entries:
      ("full", s, 0, None)        -- whole [ST, TB] tile valid
      ("diag", s, t_lo, mid)      -- cols [0,t_lo) invalid, [t_lo,t_lo+128)
                                     masked by masks128[mid], rest valid;
                                     only cols [t_lo, TB) are computed
      ("wide", s, 0, mid)         -- generic fallback, full-width mask
    masks128: [n1, ST, 128] f32; masksw: [n2, ST, TB] f32.
    """
    plan = []
    uniq128: dict = {}
    tiles128 = []
    uniqw: dict = {}
    tilesw = []

    def dedup(tile, uniq, tiles):
        key = tile.tobytes()
        mid = uniq.get(key)
        if mid is None:
            mid = len(tiles)
            uniq[key] = mid
            tiles.append(tile)
        return mid

    for tb in range(NTB):
        sub_t = mask2d[tb * TB : (tb + 1) * TB]  # [TB, T]
        entries = []
        for s in range(NST):
            sub = sub_t[:, s * ST : (s + 1) * ST]  # [TB(t), ST(s)]
            if sub.all():
                entries.append(("full", s, 0, None))
                continue
            if not sub.any():
                continue
            av = sub.any(axis=1)
            af = sub.all(axis=1)
            t_lo = 128 * (int(np.argmax(av)) // 128)
            if (
                not av[:t_lo].any()
                and af[t_lo + 128 :].all()
                and not (entries == [] and t_lo > 0)
            ):
                chunk = np.ascontiguousarray(
                    sub[t_lo : t_lo + 128].T.astype(np.float32)
                )  # [s, 128(t)]
                mid = dedup(chunk, uniq128, tiles128)
                entries.append(("diag", s, t_lo, mid))
            else:
                tile_m = np.ascontiguousarray(sub.T.astype(np.float32))
                mid = dedup(tile_m, uniqw, tilesw)
                entries.append(("wide", s, 0, mid))
        plan.append(tuple(entries))
    masks128 = (
        np.stack(tiles128) if tiles128 else np.zeros((0, ST, 128), dtype=np.float32)
    )
    masksw = (
        np.stack(tilesw) if tilesw else np.zeros((0, ST, TB), dtype=np.float32)
    )
    return tuple(plan), masks128, masksw


def _build(plan, n_m128, n_mw):
    import concourse.bacc as bacc
    import concourse.mybir as mybir
    import concourse.tile as tile

    F32 = mybir.dt.float32
    BF16 = mybir.dt.bfloat16
    Exp = mybir.ActivationFunctionType.Exp
    Ln = mybir.ActivationFunctionType.Ln

    nc = bacc.Bacc()

    xT_d = nc.declare_dram_parameter("xT", [C, T], BF16, isOutput=False)
    wq_d = nc.declare_dram_parameter("wq", [C, G * HEAD_DIM], BF16, isOutput=False)
    wk_d = nc.declare_dram_parameter("wk", [C, HEAD_DIM], BF16, isOutput=False)
    wv_d = nc.declare_dram_parameter("wv", [C, HEAD_DIM], BF16, isOutput=False)
    wo_d = nc.declare_dram_parameter("wo", [G * HEAD_DIM, C], BF16, isOutput=False)
    on_d = nc.declare_dram_parameter("ones", [128, 128], BF16, isOutput=False)
    id_d = nc.declare_dram_parameter("ident", [128, 128], BF16, isOutput=False)
    ct_d = nc.declare_dram_parameter("ctab", [HEAD_DIM, T], F32, isOutput=False)
    st_d = nc.declare_dram_parameter("stab", [HEAD_DIM, T], F32, isOutput=False)
    if n_m128:
        mk_d = nc.declare_dram_parameter(
            "masks", [n_m128 * ST, 128], BF16, isOutput=False
        )
    if n_mw:
        mkw_d = nc.declare_dram_parameter(
            "masksw", [n_mw * ST, TB], BF16, isOutput=False
        )
    out_d = nc.declare_dram_parameter("out", [T, C], BF16, isOutput=True)

    # round-robin across DMA queues in consumption order: sync + scalar are
    # the two HWDGE queues, gpsimd is the software-DGE queue
    hw_eng = [None, None]
    all_eng = [None, None, None]

    with tile.TileContext(nc) as tc:
        hw_eng[0], hw_eng[1] = nc.sync, nc.scalar
        all_eng[0], all_eng[1], all_eng[2] = nc.sync, nc.scalar, nc.gpsimd

        const = tc.alloc_tile_pool(name="const", bufs=1)
        qkv = tc.alloc_tile_pool(name="qkv", bufs=1)
        wop = tc.alloc_tile_pool(name="wop", bufs=1)

        # --- consts via the gpsimd SWDGE queue (out of the hot queues);
        # ident/ctab/stab lead (needed by the first RoPE/transpose ~20us in)
        ident = const.tile([128, 128], BF16, name="ident")
        nc.gpsimd.dma_start(out=ident, in_=id_d.ap())
        ctab = const.tile([HEAD_DIM, T], F32, name="ctab")
        stab = const.tile([HEAD_DIM, T], F32, name="stab")
        nc.gpsimd.dma_start(out=ctab, in_=ct_d.ap())
        nc.gpsimd.dma_start(out=stab, in_=st_d.ap())
        ones_sb = const.tile([128, 128], BF16, name="ones_sb")
        if n_m128:
            msk_sb = const.tile([ST, n_m128 * 128], BF16, name="msk_sb")
        if n_mw:
            mskw_sb = const.tile([ST, n_mw * TB], BF16, name="mskw_sb")

        qT = [qkv.tile([128, T], BF16, name=f"qT{h}") for h in range(G)]
        kT = qkv.tile([128, T], BF16, name="kT")
        vT = qkv.tile([128, T], BF16, name="vT")
        vch = [qkv.tile([128, 128], BF16, name=f"v{s}") for s in range(NST)]
        wo_sb = [wop.tile([128, C], BF16, name=f"wo{h}") for h in range(G)]

        # ---- phase 1: projections + RoPE + V transpose, per t-block ----
        wpool = tc.alloc_tile_pool(name="wpool", bufs=1)
        xs = tc.alloc_tile_pool(name="xs", bufs=6)
        rp = tc.alloc_tile_pool(name="rp", bufs=2)
        p1ps = tc.alloc_tile_pool(name="p1ps", bufs=1, space="PSUM")

        wq_sb = [wpool.tile([128, G * HEAD_DIM], BF16, name=f"wq{i}") for i in range(NCT)]
        wk_sb = [wpool.tile([128, HEAD_DIM], BF16, name=f"wk{i}") for i in range(NCT)]
        wv_sb = [wpool.tile([128, HEAD_DIM], BF16, name=f"wv{i}") for i in range(NCT)]

        # interleave weight-chunk and first-block xT loads across both HW
        # queues in the order phase 1 consumes them
        xt0 = []
        for ci in range(NCT):
            sl = slice(ci * 128, (ci + 1) * 128)
            e = hw_eng[ci % 2]
            e.dma_start(out=wq_sb[ci], in_=wq_d.ap()[sl, :])
            e.dma_start(out=wk_sb[ci], in_=wk_d.ap()[sl, :])
            e.dma_start(out=wv_sb[ci], in_=wv_d.ap()[sl, :])
            xt = xs.tile([128, TB], BF16, name="xt", tag="xt")
            hw_eng[(ci + 1) % 2].dma_start(out=xt, in_=xT_d.ap()[sl, 0:TB])
            xt0.append(xt)
        # phase-2 consts follow on the SWDGE queue (needed from ~60us on)
        nc.gpsimd.dma_start(out=ones_sb, in_=on_d.ap())
        if n_m128:
            for i in range(n_m128):
                nc.gpsimd.dma_start(
                    out=msk_sb[:, i * 128 : (i + 1) * 128],
                    in_=mk_d.ap()[i * ST : (i + 1) * ST, :],
                )
        if n_mw:
            for i in range(n_mw):
                nc.gpsimd.dma_start(
                    out=mskw_sb[:, i * TB : (i + 1) * TB],
                    in_=mkw_d.ap()[i * ST : (i + 1) * ST, :],
                )

        def rope(src_f32, dst_region, tsl):
            swp = rp.tile([128, TB], F32, name="swp", tag="swp")
            t1 = rp.tile([128, TB], F32, name="t1", tag="t1")
            nc.vector.stream_shuffle(swp, src_f32, SWAP_MASK)
            nc.vector.tensor_mul(t1, src_f32, ctab[:, tsl])
            nc.vector.tensor_mul(swp, swp, stab[:, tsl])
            nc.vector.tensor_add(dst_region, t1, swp)

        for tb in range(NTB):
            tsl = slice(tb * TB, (tb + 1) * TB)
            q_ps = [
                p1ps.tile(
                    [128, TB], F32, name=f"qps{h}", tag=f"qps{h}",
                    bufs=2 if h == 0 else 1,
                )
                for h in range(G)
            ]
            k_ps = p1ps.tile([128, TB], F32, name="kps", tag="kps")
            v_ps = p1ps.tile([128, TB], F32, name="vps", tag="vps")
            for ci in range(NCT):
                if tb == 0:
                    xt = xt0[ci]
                else:
                    xt = xs.tile([128, TB], BF16, name="xt", tag="xt")
                    hw_eng[(ci + tb) % 2].dma_start(
                        out=xt, in_=xT_d.ap()[ci * 128 : (ci + 1) * 128, tsl]
                    )
                first, last = ci == 0, ci == NCT - 1
                for h in range(G):
                    nc.tensor.matmul(
                        q_ps[h],
                        lhsT=wq_sb[ci][:, h * 128 : (h + 1) * 128],
                        rhs=xt,
                        start=first,
                        stop=last,
                    )
                nc.tensor.matmul(
                    k_ps, lhsT=wk_sb[ci], rhs=xt, start=first, stop=last
                )
                nc.tensor.matmul(
                    v_ps, lhsT=wv_sb[ci], rhs=xt, start=first, stop=last
                )
            # drain PSUM fast via ACT (fp32 staging for rope; V direct)
            for h in range(G):
                qf = rp.tile([128, TB], F32, name=f"qf{h}", tag=f"qf{h}")
                nc.scalar.copy(qf, q_ps[h])
                rope(qf, qT[h][:, tsl], tsl)
            kf = rp.tile([128, TB], F32, name="kf", tag="kf")
            nc.scalar.copy(kf, k_ps)
            rope(kf, kT[:, tsl], tsl)
            nc.scalar.copy(vT[:, tsl], v_ps)
            # transpose this block's V chunks to natural [s, d] layout
            for sc in range(TB // 128):
                s = tb * (TB // 128) + sc
                vtp = p1ps.tile([128, 128], BF16, name="vtp", tag="vtp")
                nc.tensor.transpose(
                    vtp, vT[:, s * 128 : (s + 1) * 128], ident
                )
                nc.scalar.copy(vch[s], vtp)

        # wo loads after all phase-1 traffic (needed only from o_proj on)
        for h in range(G):
            hw_eng[h % 2].dma_start(
                out=wo_sb[h], in_=wo_d.ap()[h * 128 : (h + 1) * 128, :]
            )

        p1ps.release()
        rp.release()
        xs.release()
        wpool.release()

        # ---- phases 2+3, software-pipelined: o_proj(tb) after scores(tb') ----
        p2sb = tc.alloc_tile_pool(name="p2sb", bufs=3)
        p2ps = tc.alloc_tile_pool(name="p2ps", bufs=1, space="PSUM")
        outp = tc.alloc_tile_pool(name="outp", bufs=2)
        p3sb = tc.alloc_tile_pool(name="p3sb", bufs=4)

        oTu = {}  # (tb) -> list of 4 normalized bf16 tiles
        dma_ctr = [0]
        Div = mybir.AluOpType.divide

        def scores_block(tb):
            tsl = slice(tb * TB, (tb + 1) * TB)
            entries = plan[tb]
            tiles = []
            for h in range(G):
                oTu_t = outp.tile([128, TB], BF16, name=f"oTu{h}", tag=f"oTu{h}")
                tiles.append(oTu_t)
                if not entries:
                    nc.gpsimd.memset(oTu_t, 0.0)
                    continue
                oT_ps = p2ps.tile([128, TB], F32, name="oTps", tag="oTps", bufs=2)
                # den broadcast across all 128 partitions (wide-ones lhsT)
                den = p2ps.tile([128, TB], F32, name="den", tag="den", bufs=2)
                n_e = len(entries)
                eps = []
                for idx, (kind, s, t_lo, mid) in enumerate(entries):
                    col = slice(t_lo, TB)  # computed column range
                    stp = p2ps.tile([128, TB], F32, name="stp", tag="stp", bufs=2)
                    nc.tensor.matmul(
                        stp[:, col],
                        lhsT=kT[:, s * 128 : (s + 1) * 128],
                        rhs=qT[h][:, tb * TB + t_lo : (tb + 1) * TB],
                        start=True,
                        stop=True,
                    )
                    ep = p2sb.tile([ST, TB], BF16, name="ep", tag="ep", bufs=18)
                    eps.append((ep, col))
                    nc.scalar.activation(ep[:, col], stp[:, col], Exp, scale=SCALE)
                    if kind == "diag" and mid is not None:
                        nc.gpsimd.tensor_mul(
                            ep[:, t_lo : t_lo + 128],
                            ep[:, t_lo : t_lo + 128],
                            msk_sb[:, mid * 128 : (mid + 1) * 128],
                        )
                    elif kind == "wide":
                        nc.gpsimd.tensor_mul(
                            ep, ep, mskw_sb[:, mid * TB : (mid + 1) * TB]
                        )
                    first, last = idx == 0, idx == n_e - 1
                    nc.tensor.matmul(
                        oT_ps[:, col], lhsT=vch[s], rhs=ep[:, col],
                        start=first, stop=last,
                    )
                # den matmuls batched after the s-loop: one shared lhsT
                for idx, (ep, col) in enumerate(eps):
                    nc.tensor.matmul(
                        den[:, col], lhsT=ones_sb, rhs=ep[:, col],
                        start=idx == 0, stop=idx == n_e - 1,
                    )
                # 1/den on DVE (den already broadcast on all partitions via
                # the wide-ones lhsT); then one DVE mul normalizes, drains
                # PSUM and casts to bf16
                rcp_b = p2sb.tile([128, TB], F32, name="rcpb", tag="rcpb", bufs=2)
                nc.vector.reciprocal_approx_fast(rcp_b, den)
                nc.vector.tensor_mul(tiles[h], oT_ps, rcp_b)
            oTu[tb] = tiles

        def finish_block(tb, last=False):
            tiles = oTu.pop(tb)
            for cb in range(C // 512):
                for tch in range(TB // 128):
                    ops = p2ps.tile([128, 512], F32, name="ops", tag="ops", bufs=2)
                    for h in range(G):
                        nc.tensor.matmul(
                            ops,
                            lhsT=tiles[h][:, tch * 128 : (tch + 1) * 128],
                            rhs=wo_sb[h][:, cb * 512 : (cb + 1) * 512],
                            start=h == 0,
                            stop=h == G - 1,
                        )
                    osb = p3sb.tile([128, 512], BF16, name="osb", tag="osb", bufs=12)
                    if dma_ctr[0] % 2 == 0:
                        nc.scalar.copy(osb, ops)
                    else:
                        nc.vector.tensor_copy(osb, ops)
                    t0 = tb * TB + tch * 128
                    eng = (
                        all_eng[dma_ctr[0] % 3] if last else hw_eng[dma_ctr[0] % 2]
                    )
                    eng.dma_start(
                        out=out_d.ap()[t0 : t0 + 128, cb * 512 : (cb + 1) * 512],
                        in_=osb,
                    )
                    dma_ctr[0] += 1

        order = [NTB - 1 - i for i in range(NTB)]  # big blocks first
        prev = None
        for tb in order:
            scores_block(tb)
            if prev is not None:
                finish_block(prev)
            prev = tb
        finish_block(prev, last=True)

        p3sb.release()
        outp.release()
        p2ps.release()
        p2sb.release()
        wop.release()
        qkv.release()
        const.release()

    nc.compile()
    return nc


def _to_bf16(a):
    import ml_dtypes

    return np.ascontiguousarray(np.asarray(a, dtype=np.float32)).astype(
        ml_dtypes.bfloat16
    )


def _prep_inputs(x, cos, sin, Wq, Wk, Wv, Wo, masks128, masksw):
    cos = np.asarray(cos, dtype=np.float32).reshape(T, HEAD_DIM // 2)
    sin = np.asarray(sin, dtype=np.float32).reshape(T, HEAD_DIM // 2)
    ctab = np.ascontiguousarray(np.repeat(cos, 2, axis=1).T)  # [128, T]
    s2 = np.repeat(sin, 2, axis=1)
    s2[:, 0::2] *= -1.0
    stab = np.ascontiguousarray(s2.T)

    xTb = [_to_bf16(np.asarray(x[b], dtype=np.float32).T) for b in range(B)]
    in_maps = []
    for core in range(8):
        b, g = divmod(core, NUM_KV_HEADS)
        m = {
            "xT": xTb[b],
            "wq": _to_bf16(Wq[:, g * 512 : (g + 1) * 512]),
            "wk": _to_bf16(Wk[:, g * 128 : (g + 1) * 128]),
            "wv": _to_bf16(Wv[:, g * 128 : (g + 1) * 128]),
            "wo": _to_bf16(Wo[g * 512 : (g + 1) * 512, :]),
            "ctab": ctab,
            "stab": stab,
            "ones": _to_bf16(np.ones((128, 128), dtype=np.float32)),
            "ident": _to_bf16(np.eye(128, dtype=np.float32)),
        }
        if masks128.shape[0]:
            m["masks"] = _to_bf16(masks128.reshape(-1, 128))
        if masksw.shape[0]:
            m["masksw"] = _to_bf16(masksw.reshape(-1, TB))
        in_maps.append(m)
    return in_maps


def kernel(x, cos, sin, mask, Wq, Wk, Wv, Wo, _trace=False, _result_box=None):
    from concourse.bass_utils import run_bass_kernel_spmd

    mask2d = np.asarray(mask).reshape(T, T).astype(bool)
    plan, masks128, masksw = _classify_mask(mask2d)

    key = (plan, masks128.shape[0], masksw.shape[0])
    nc = _nc_cache.get(key)
    if nc is None:
        nc = _build(plan, masks128.shape[0], masksw.shape[0])
        _nc_cache[key] = nc

    in_maps = _prep_inputs(x, cos, sin, Wq, Wk, Wv, Wo, masks128, masksw)
    res = run_bass_kernel_spmd(nc, in_maps, core_ids=list(range(8)), trace=_trace)
    if _result_box is not None:
        _result_box.append(res)

    out = np.zeros((B, T, C), dtype=np.float32)
    for core in range(8):
        b = core // NUM_KV_HEADS
        out[b] += np.asarray(res.results[core]["out"], dtype=np.float32)
    return out


# revision 51
# speedup vs baseline: 1.0280x; 1.0280x over previous
"""Grouped-Query Attention (B=2, T=2048, C=2048, 16 Q heads / 4 KV heads,
D=128) on 8 Trainium2 NeuronCores.

Sharding: core (b, g) for b in {0,1}, g in {0..3} handles batch b and KV head
g (= query heads 4g..4g+3). Each core computes its 4 heads' attention plus the
partial output projection against its 512-row slice of Wo; the host sums the
4 partials per batch (the "all-reduce" of the o_proj, done in numpy).

v2 (this file) vs baseline:
  * all matmul operands bf16 (inputs staged bf16 on host); PSUM stays fp32.
  * DMA spread across both HWDGE queues (sync + scalar engines) interleaved
    in consumption order, consts on the gpsimd SWDGE queue; kills the 56us
    startup stall and the single-queue DMA saturation.
  * RoPE runs per-t-block inside phase 1 (overlapped with projections of the
    next block) instead of as a serial phase.
  * softmax denominators batched per t-block: dens copied to one [4,512]
    sbuf tile, ONE reciprocal per t-block (was 16 x 3.3us on DVE), rescale
    via partition-broadcast AP reads (no gpsimd partition_broadcast).
  * o_proj of block tb emitted after scores of block tb' (software pipeline)
    so the PE never waits for the softmax tail.
"""
import sys

sys.path.insert(0, "/opt/trn_rl_repo")

import numpy as np

B, T, C = 2, 2048, 2048
NUM_HEADS, NUM_KV_HEADS, HEAD_DIM = 16, 4, 128
G = NUM_HEADS // NUM_KV_HEADS  # 4 query heads per core
SCALE = float(HEAD_DIM) ** -0.5
TB = 512  # t-block (matmul moving free dim)
NTB = T // TB  # 4
ST = 128  # s-tile
NST = T // ST  # 16
NCT = C // 128  # 16 contraction tiles

SWAP_MASK = [i ^ 1 for i in range(32)]

_nc_cache: dict = {}


def _classify_mask(mask2d: np.ndarray):
    """mask2d[t, s] bool. Returns (plan, masks128, masksw).

    plan[tb] is a tuple of entries:
      ("full", s, 0, None)        -- whole [ST, TB] tile valid
      ("diag", s, t_lo, mid)      -- cols [0,t_lo) invalid, [t_lo,t_lo+128)
                                     masked by masks128[mid], rest valid;
                                     only cols [t_lo, TB) are computed
      ("wide", s, 0, mid)         -- generic fallback, full-width mask
    masks128: [n1, ST, 128] f32; masksw: [n2, ST, TB] f32.
    """
    plan = []
    uniq128: dict = {}
    tiles128 = []
    uniqw: dict = {}
    tilesw = []

    def dedup(tile, uniq, tiles):
        key = tile.tobytes()
        mid = uniq.get(key)
        if mid is None:
            mid = len(tiles)
            uniq[key] = mid
            tiles.append(tile)
        return mid

    for tb in range(NTB):
        sub_t = mask2d[tb * TB : (tb + 1) * TB]  # [TB, T]
        entries = []
        for s in range(NST):
            sub = sub_t[:, s * ST : (s + 1) * ST]  # [TB(t), ST(s)]
            if sub.all():
                entries.append(("full", s, 0, None))
                continue
            if not sub.any():
                continue
            av = sub.any(axis=1)
            af = sub.all(axis=1)
            t_lo = 128 * (int(np.argmax(av)) // 128)
            if (
                not av[:t_lo].any()
                and af[t_lo + 128 :].all()
                and not (entries == [] and t_lo > 0)
            ):
                chunk = np.ascontiguousarray(
                    sub[t_lo : t_lo + 128].T.astype(np.float32)
                )  # [s, 128(t)]
                mid = dedup(chunk, uniq128, tiles128)
                entries.append(("diag", s, t_lo, mid))
            else:
                tile_m = np.ascontiguousarray(sub.T.astype(np.float32))
                mid = dedup(tile_m, uniqw, tilesw)
                entries.append(("wide", s, 0, mid))
        plan.append(tuple(entries))
    masks128 = (
        np.stack(tiles128) if tiles128 else np.zeros((0, ST, 128), dtype=np.float32)
    )
    masksw = (
        np.stack(tilesw) if tilesw else np.zeros((0, ST, TB), dtype=np.float32)
    )
    return tuple(plan), masks128, masksw


def _build(plan, n_m128, n_mw):
    import concourse.bacc as bacc
    import concourse.mybir as mybir
    import concourse.tile as tile

    F32 = mybir.dt.float32
    BF16 = mybir.dt.bfloat16
    Exp = mybir.ActivationFunctionType.Exp
    Ln = mybir.ActivationFunctionType.Ln

    nc = bacc.Bacc()

    xT_d = nc.declare_dram_parameter("xT", [C, T], BF16, isOutput=False)
    wq_d = nc.declare_dram_parameter("wq", [C, G * HEAD_DIM], BF16, isOutput=False)
    wk_d = nc.declare_dram_parameter("wk", [C, HEAD_DIM], BF16, isOutput=False)
    wv_d = nc.declare_dram_parameter("wv", [C, HEAD_DIM], BF16, isOutput=False)
    wo_d = nc.declare_dram_parameter("wo", [G * HEAD_DIM, C], BF16, isOutput=False)
    on_d = nc.declare_dram_parameter("ones", [128, 128], BF16, isOutput=False)
    id_d = nc.declare_dram_parameter("ident", [128, 128], BF16, isOutput=False)
    ct_d = nc.declare_dram_parameter("ctab", [HEAD_DIM, T], F32, isOutput=False)
    st_d = nc.declare_dram_parameter("stab", [HEAD_DIM, T], F32, isOutput=False)
    if n_m128:
        mk_d = nc.declare_dram_parameter(
            "masks", [n_m128 * ST, 128], BF16, isOutput=False
        )
    if n_mw:
        mkw_d = nc.declare_dram_parameter(
            "masksw", [n_mw * ST, TB], BF16, isOutput=False
        )
    out_d = nc.declare_dram_parameter("out", [T, C], BF16, isOutput=True)

    # round-robin across DMA queues in consumption order: sync + scalar are
    # the two HWDGE queues, gpsimd is the software-DGE queue
    hw_eng = [None, None]
    all_eng = [None, None, None]

    with tile.TileContext(nc) as tc:
        hw_eng[0], hw_eng[1] = nc.sync, nc.scalar
        all_eng[0], all_eng[1], all_eng[2] = nc.sync, nc.scalar, nc.gpsimd

        const = tc.alloc_tile_pool(name="const", bufs=1)
        qkv = tc.alloc_tile_pool(name="qkv", bufs=1)
        wop = tc.alloc_tile_pool(name="wop", bufs=1)

        # --- consts via the gpsimd SWDGE queue (out of the hot queues);
        # their DMAs are issued after the phase-1 K/V weight chunks below
        ident = const.tile([128, 128], BF16, name="ident")
        ctab = const.tile([HEAD_DIM, T], F32, name="ctab")
        stab = const.tile([HEAD_DIM, T], F32, name="stab")
        ones_sb = const.tile([128, 128], BF16, name="ones_sb")
        if n_m128:
            msk_sb = const.tile([ST, n_m128 * 128], BF16, name="msk_sb")
        if n_mw:
            mskw_sb = const.tile([ST, n_mw * TB], BF16, name="mskw_sb")

        qT = [qkv.tile([128, T], BF16, name=f"qT{h}") for h in range(G)]
        kT = qkv.tile([128, T], BF16, name="kT")
        vT = qkv.tile([128, T], BF16, name="vT")
        vch = [qkv.tile([128, 128], BF16, name=f"v{s}") for s in range(NST)]
        wo_sb = [wop.tile([128, C], BF16, name=f"wo{h}") for h in range(G)]

        # ---- phase 1: projections + RoPE + V transpose, per t-block ----
        wpool = tc.alloc_tile_pool(name="wpool", bufs=1)
        xs = tc.alloc_tile_pool(name="xs", bufs=6)
        rp = tc.alloc_tile_pool(name="rp", bufs=2)
        p1ps = tc.alloc_tile_pool(name="p1ps", bufs=1, space="PSUM")

        wq_sb = [wpool.tile([128, G * HEAD_DIM], BF16, name=f"wq{i}") for i in range(NCT)]
        wk_sb = [wpool.tile([128, HEAD_DIM], BF16, name=f"wk{i}") for i in range(NCT)]
        wv_sb = [wpool.tile([128, HEAD_DIM], BF16, name=f"wv{i}") for i in range(NCT)]

        # interleave weight-chunk and first-block xT loads across both HW
        # queues in the order phase 1 consumes them
        xt0 = []
        for ci in range(NCT):
            sl = slice(ci * 128, (ci + 1) * 128)
            e = hw_eng[ci % 2]
            e.dma_start(out=wq_sb[ci], in_=wq_d.ap()[sl, :])
            # K/V weight chunks ride the SWDGE queue: relieves the two HW
            # queues during the DMA-gated first t-block
            nc.gpsimd.dma_start(out=wk_sb[ci], in_=wk_d.ap()[sl, :])
            nc.gpsimd.dma_start(out=wv_sb[ci], in_=wv_d.ap()[sl, :])
            xt = xs.tile([128, TB], BF16, name="xt", tag="xt")
            hw_eng[(ci + 1) % 2].dma_start(out=xt, in_=xT_d.ap()[sl, 0:TB])
            xt0.append(xt)
        # RoPE/transpose consts next (needed ~20us in), then phase-2 consts
        nc.gpsimd.dma_start(out=ident, in_=id_d.ap())
        nc.gpsimd.dma_start(out=ctab, in_=ct_d.ap())
        nc.gpsimd.dma_start(out=stab, in_=st_d.ap())
        nc.gpsimd.dma_start(out=ones_sb, in_=on_d.ap())
        if n_m128:
            for i in range(n_m128):
                nc.gpsimd.dma_start(
                    out=msk_sb[:, i * 128 : (i + 1) * 128],
                    in_=mk_d.ap()[i * ST : (i + 1) * ST, :],
                )
        if n_mw:
            for i in range(n_mw):
                nc.gpsimd.dma_start(
                    out=mskw_sb[:, i * TB : (i + 1) * TB],
                    in_=mkw_d.ap()[i * ST : (i + 1) * ST, :],
                )

        def rope(src_f32, dst_region, tsl):
            swp = rp.tile([128, TB], F32, name="swp", tag="swp")
            t1 = rp.tile([128, TB], F32, name="t1", tag="t1")
            nc.vector.stream_shuffle(swp, src_f32, SWAP_MASK)
            nc.vector.tensor_mul(t1, src_f32, ctab[:, tsl])
            nc.vector.tensor_mul(swp, swp, stab[:, tsl])
            nc.vector.tensor_add(dst_region, t1, swp)

        for tb in range(NTB):
            tsl = slice(tb * TB, (tb + 1) * TB)
            q_ps = [
                p1ps.tile(
                    [128, TB], F32, name=f"qps{h}", tag=f"qps{h}",
                    bufs=2 if h == 0 else 1,
                )
                for h in range(G)
            ]
            k_ps = p1ps.tile([128, TB], F32, name="kps", tag="kps")
            v_ps = p1ps.tile([128, TB], F32, name="vps", tag="vps")
            for ci in range(NCT):
                if tb == 0:
                    xt = xt0[ci]
                else:
                    xt = xs.tile([128, TB], BF16, name="xt", tag="xt")
                    hw_eng[(ci + tb) % 2].dma_start(
                        out=xt, in_=xT_d.ap()[ci * 128 : (ci + 1) * 128, tsl]
                    )
                first, last = ci == 0, ci == NCT - 1
                for h in range(G):
                    nc.tensor.matmul(
                        q_ps[h],
                        lhsT=wq_sb[ci][:, h * 128 : (h + 1) * 128],
                        rhs=xt,
                        start=first,
                        stop=last,
                    )
                nc.tensor.matmul(
                    k_ps, lhsT=wk_sb[ci], rhs=xt, start=first, stop=last
                )
                nc.tensor.matmul(
                    v_ps, lhsT=wv_sb[ci], rhs=xt, start=first, stop=last
                )
            # drain PSUM fast: V on DVE (parallel with ACT's Q/K staging)
            nc.vector.tensor_copy(vT[:, tsl], v_ps)
            for h in range(G):
                qf = rp.tile([128, TB], F32, name=f"qf{h}", tag=f"qf{h}")
                nc.scalar.copy(qf, q_ps[h])
                rope(qf, qT[h][:, tsl], tsl)
            kf = rp.tile([128, TB], F32, name="kf", tag="kf")
            nc.scalar.copy(kf, k_ps)
            rope(kf, kT[:, tsl], tsl)
            # transpose this block's V chunks to natural [s, d] layout
            for sc in range(TB // 128):
                s = tb * (TB // 128) + sc
                vtp = p1ps.tile([128, 128], BF16, name="vtp", tag="vtp")
                nc.tensor.transpose(
                    vtp, vT[:, s * 128 : (s + 1) * 128], ident
                )
                nc.scalar.copy(vch[s], vtp)

        # wo loads after all phase-1 traffic (needed only from o_proj on)
        for h in range(G):
            hw_eng[h % 2].dma_start(
                out=wo_sb[h], in_=wo_d.ap()[h * 128 : (h + 1) * 128, :]
            )

        p1ps.release()
        rp.release()
        xs.release()
        wpool.release()

        # ---- phases 2+3, software-pipelined: o_proj(tb) after scores(tb') ----
        p2sb = tc.alloc_tile_pool(name="p2sb", bufs=3)
        p2ps = tc.alloc_tile_pool(name="p2ps", bufs=1, space="PSUM")
        outp = tc.alloc_tile_pool(name="outp", bufs=2)
        p3sb = tc.alloc_tile_pool(name="p3sb", bufs=4)

        oTu = {}  # (tb) -> list of 4 normalized bf16 tiles
        dma_ctr = [0]
        Div = mybir.AluOpType.divide

        def scores_block(tb):
            tsl = slice(tb * TB, (tb + 1) * TB)
            entries = plan[tb]
            tiles = []
            for h in range(G):
                oTu_t = outp.tile([128, TB], BF16, name=f"oTu{h}", tag=f"oTu{h}")
                tiles.append(oTu_t)
                if not entries:
                    nc.gpsimd.memset(oTu_t, 0.0)
                    continue
                oT_ps = p2ps.tile([128, TB], F32, name="oTps", tag="oTps", bufs=2)
                # den broadcast across all 128 partitions (wide-ones lhsT)
                den = p2ps.tile([128, TB], F32, name="den", tag="den", bufs=2)
                n_e = len(entries)
                eps = []
                for idx, (kind, s, t_lo, mid) in enumerate(entries):
                    col = slice(t_lo, TB)  # computed column range
                    stp = p2ps.tile([128, TB], F32, name="stp", tag="stp", bufs=2)
                    nc.tensor.matmul(
                        stp[:, col],
                        lhsT=kT[:, s * 128 : (s + 1) * 128],
                        rhs=qT[h][:, tb * TB + t_lo : (tb + 1) * TB],
                        start=True,
                        stop=True,
                    )
                    ep = p2sb.tile([ST, TB], BF16, name="ep", tag="ep", bufs=18)
                    eps.append((ep, col))
                    nc.scalar.activation(ep[:, col], stp[:, col], Exp, scale=SCALE)
                    if kind == "diag" and mid is not None:
                        nc.gpsimd.tensor_mul(
                            ep[:, t_lo : t_lo + 128],
                            ep[:, t_lo : t_lo + 128],
                            msk_sb[:, mid * 128 : (mid + 1) * 128],
                        )
                    elif kind == "wide":
                        nc.gpsimd.tensor_mul(
                            ep, ep, mskw_sb[:, mid * TB : (mid + 1) * TB]
                        )
                    first, last = idx == 0, idx == n_e - 1
                    nc.tensor.matmul(
                        oT_ps[:, col], lhsT=vch[s], rhs=ep[:, col],
                        start=first, stop=last,
                    )
                # den matmuls batched after the s-loop: one shared lhsT
                for idx, (ep, col) in enumerate(eps):
                    nc.tensor.matmul(
                        den[:, col], lhsT=ones_sb, rhs=ep[:, col],
                        start=idx == 0, stop=idx == n_e - 1,
                    )
                # 1/den on DVE (den already broadcast on all partitions via
                # the wide-ones lhsT); then one DVE mul normalizes, drains
                # PSUM and casts to bf16
                rcp_b = p2sb.tile([128, TB], F32, name="rcpb", tag="rcpb", bufs=2)
                nc.vector.reciprocal_approx_fast(rcp_b, den)
                nc.vector.tensor_mul(tiles[h], oT_ps, rcp_b)
            oTu[tb] = tiles

        def finish_chunks(tb, last=False):
            tiles = oTu.pop(tb)
            for cb in range(C // 512):
                for tch in range(TB // 128):
                    ops = p2ps.tile([128, 512], F32, name="ops", tag="ops", bufs=2)
                    for h in range(G):
                        nc.tensor.matmul(
                            ops,
                            lhsT=tiles[h][:, tch * 128 : (tch + 1) * 128],
                            rhs=wo_sb[h][:, cb * 512 : (cb + 1) * 512],
                            start=h == 0,
                            stop=h == G - 1,
                        )
                    osb = p3sb.tile([128, 512], BF16, name="osb", tag="osb", bufs=12)
                    if dma_ctr[0] % 2 == 0:
                        nc.scalar.copy(osb, ops)
                    else:
                        nc.vector.tensor_copy(osb, ops)
                    t0 = tb * TB + tch * 128
                    eng = (
                        all_eng[dma_ctr[0] % 3] if last else hw_eng[dma_ctr[0] % 2]
                    )
                    eng.dma_start(
                        out=out_d.ap()[t0 : t0 + 128, cb * 512 : (cb + 1) * 512],
                        in_=osb,
                    )
                    dma_ctr[0] += 1
                    yield

        order = [NTB - 1 - i for i in range(NTB)]  # big blocks first
        prev = None
        for i, tb in enumerate(order):
            scores_block(tb)
            if prev is not None and i < len(order) - 1:
                for _ in finish_chunks(prev):
                    pass
            prev = tb
        # interleave the last two o_proj blocks chunk-wise so the final
        # block's output DMA starts draining ~10us earlier
        import itertools

        gens = [finish_chunks(t, last=True) for t in (order[-2], order[-1])]
        for _ in itertools.zip_longest(*gens):
            pass

        p3sb.release()
        outp.release()
        p2ps.release()
        p2sb.release()
        wop.release()
        qkv.release()
        const.release()

    nc.compile()
    return nc


def _to_bf16(a):
    import ml_dtypes

    return np.ascontiguousarray(np.asarray(a, dtype=np.float32)).astype(
        ml_dtypes.bfloat16
    )


def _prep_inputs(x, cos, sin, Wq, Wk, Wv, Wo, masks128, masksw):
    cos = np.asarray(cos, dtype=np.float32).reshape(T, HEAD_DIM // 2)
    sin = np.asarray(sin, dtype=np.float32).reshape(T, HEAD_DIM // 2)
    ctab = np.ascontiguousarray(np.repeat(cos, 2, axis=1).T)  # [128, T]
    s2 = np.repeat(sin, 2, axis=1)
    s2[:, 0::2] *= -1.0
    stab = np.ascontiguousarray(s2.T)

    xTb = [_to_bf16(np.asarray(x[b], dtype=np.float32).T) for b in range(B)]
    in_maps = []
    for core in range(8):
        b, g = divmod(core, NUM_KV_HEADS)
        m = {
            "xT": xTb[b],
            "wq": _to_bf16(Wq[:, g * 512 : (g + 1) * 512]),
            "wk": _to_bf16(Wk[:, g * 128 : (g + 1) * 128]),
            "wv": _to_bf16(Wv[:, g * 128 : (g + 1) * 128]),
            "wo": _to_bf16(Wo[g * 512 : (g + 1) * 512, :]),
            "ctab": ctab,
            "stab": stab,
            "ones": _to_bf16(np.ones((128, 128), dtype=np.float32)),
            "ident": _to_bf16(np.eye(128, dtype=np.float32)),
        }
        if masks128.shape[0]:
            m["masks"] = _to_bf16(masks128.reshape(-1, 128))
        if masksw.shape[0]:
            m["masksw"] = _to_bf16(masksw.reshape(-1, TB))
        in_maps.append(m)
    return in_maps


def kernel(x, cos, sin, mask, Wq, Wk, Wv, Wo, _trace=False, _result_box=None):
    from concourse.bass_utils import run_bass_kernel_spmd

    mask2d = np.asarray(mask).reshape(T, T).astype(bool)
    plan, masks128, masksw = _classify_mask(mask2d)

    key = (plan, masks128.shape[0], masksw.shape[0])
    nc = _nc_cache.get(key)
    if nc is None:
        nc = _build(plan, masks128.shape[0], masksw.shape[0])
        _nc_cache[key] = nc

    in_maps = _prep_inputs(x, cos, sin, Wq, Wk, Wv, Wo, masks128, masksw)
    res = run_bass_kernel_spmd(nc, in_maps, core_ids=list(range(8)), trace=_trace)
    if _result_box is not None:
        _result_box.append(res)

    out = np.zeros((B, T, C), dtype=np.float32)
    for core in range(8):
        b = core // NUM_KV_HEADS
        out[b] += np.asarray(res.results[core]["out"], dtype=np.float32)
    return out
